# revision 7
# baseline (speedup 1.0000x reference)
"""Trainium2 Bass kernel: causal multi-head self-attention block (pre-LN).

Full module computed on 8 NeuronCores:
    xn = LayerNorm(x); q = xn@Wq.T+bq; k,v = xn@Wkv.T+bkv
    out = softmax(mask(q k^T / sqrt(dh))) v @ Wo.T + bo + x

Sharding: core = batch_index * 2 + head_half.  Each core handles one batch
element and 8 of the 16 heads (column-parallel QKV, row-parallel Wo), emits a
partial [S, D] output including half the residual; host sums core pairs and
adds bo.

Shapes are hardcoded for B=4, S=2048, D=1024, H=16, DH=64.
"""

import os
import sys

import numpy as np

sys.path.insert(0, "/opt/trn_rl_repo")

B, S, D, H = 4, 2048, 1024, 16
DH = D // H            # 64
HL = H // 2            # heads per core: 8
OH = HL * DH           # per-core head features: 512
EPS = 1e-5
NEG = -30000.0         # additive mask; exp(x + NEG) underflows to 0
P = 128                # SBUF partitions
NST = S // P           # 16 s-tiles
NFT = D // P           # 8 feature tiles
NOT = OH // P          # 4 o-tiles (per-core head features)
QS = 512               # query super-tile (matmul moving free dim)
NQS = S // QS          # 4
KT_PER_QS = QS // P    # 4 k-tiles per q-super

_CACHE = {}


def _build_nc():
    import concourse.bass as bass
    import concourse.bacc as bacc
    import concourse.tile as tile
    from concourse import mybir
    from concourse.masks import make_identity

    f32 = mybir.dt.float32
    bf16 = mybir.dt.bfloat16
    Alu = mybir.AluOpType
    Act = mybir.ActivationFunctionType

    nc = bacc.Bacc("TRN2", target_bir_lowering=False, debug=False, num_devices=8)

    # ---- DRAM I/O (per-core shard shapes) ----
    x_d = nc.dram_tensor("x", [S, D], f32, kind="ExternalInput").ap()
    wq_d = nc.dram_tensor("wq", [OH, D], f32, kind="ExternalInput").ap()
    wk_d = nc.dram_tensor("wk", [OH, D], f32, kind="ExternalInput").ap()
    wv_d = nc.dram_tensor("wv", [OH, D], f32, kind="ExternalInput").ap()
    wo_d = nc.dram_tensor("wo", [D, OH], f32, kind="ExternalInput").ap()
    bq_d = nc.dram_tensor("bq", [OH], f32, kind="ExternalInput").ap()
    bk_d = nc.dram_tensor("bk", [OH], f32, kind="ExternalInput").ap()
    bv_d = nc.dram_tensor("bv", [OH], f32, kind="ExternalInput").ap()
    g_d = nc.dram_tensor("ln_g", [D], f32, kind="ExternalInput").ap()
    b_d = nc.dram_tensor("ln_b", [D], f32, kind="ExternalInput").ap()
    pad_d = nc.dram_tensor("pad", [P, NST], f32, kind="ExternalInput").ap()
    out_d = nc.dram_tensor("out", [S, D], f32, kind="ExternalOutput").ap()

    def bcast(ap_1d, n):
        # [n] dram vector -> [P, n] partition-broadcast DMA source
        return bass.AP(tensor=ap_1d.tensor, offset=ap_1d.offset,
                       ap=[[0, P], [1, n]])

    with tile.TileContext(nc) as tc:
        with (
            tc.tile_pool(name="res", bufs=1) as res,       # resident tensors
            tc.tile_pool(name="small", bufs=4) as small,
        ):
            # ---------- constants ----------
            ident_f = res.tile([P, P], f32, tag="ident_f")
            make_identity(nc, ident_f)
            ident_b = res.tile([P, P], bf16, tag="ident_b")
            nc.vector.tensor_copy(ident_b, ident_f)

            g_sb = res.tile([P, D], f32, tag="g_sb")
            nc.sync.dma_start(out=g_sb, in_=bcast(g_d, D))
            b_sb = res.tile([P, D], f32, tag="b_sb")
            nc.sync.dma_start(out=b_sb, in_=bcast(b_d, D))
            vb_sb = res.tile([P, OH], f32, tag="vb_sb")
            nc.sync.dma_start(out=vb_sb, in_=bcast(bv_d, OH))
            pad_sb = res.tile([P, NST], f32, tag="pad_sb")
            nc.sync.dma_start(out=pad_sb, in_=pad_d)
            bq_sb = res.tile([P, NOT], f32, tag="bq_sb")
            nc.sync.dma_start(out=bq_sb, in_=bq_d.rearrange("(t p) -> p t", p=P))
            bk_sb = res.tile([P, NOT], f32, tag="bk_sb")
            nc.sync.dma_start(out=bk_sb, in_=bk_d.rearrange("(t p) -> p t", p=P))
            eps_sb = res.tile([P, 1], f32, tag="eps_sb")
            nc.vector.memset(eps_sb, EPS)

            # ---------- resident big tensors ----------
            xnT = [res.tile([P, S], bf16, tag=f"xnT{j}", name=f"xnT{j}") for j in range(NFT)]
            qT = [res.tile([P, S], bf16, tag=f"qT{t}", name=f"qT{t}") for t in range(NOT)]
            kT = [res.tile([P, S], bf16, tag=f"kT{t}", name=f"kT{t}") for t in range(NOT)]
            # V augmented with a ones column per head: [s, h*65 .. h*65+64]
            vaug = [res.tile([P, HL * (DH + 1)], bf16, tag=f"vaug{i}", name=f"vaug{i}")
                    for i in range(NST)]
            oT = [res.tile([P, S], bf16, tag=f"oT{t}", name=f"oT{t}") for t in range(NOT)]
            wqT = [res.tile([P, OH], bf16, tag=f"wqT{j}", name=f"wqT{j}") for j in range(NFT)]
            wkT = [res.tile([P, OH], bf16, tag=f"wkT{j}", name=f"wkT{j}") for j in range(NFT)]
            wvT = [res.tile([P, OH], bf16, tag=f"wvT{j}", name=f"wvT{j}") for j in range(NFT)]
            woT = [res.tile([P, D], bf16, tag=f"woT{t}", name=f"woT{t}") for t in range(NOT)]

            # ---------- phase A+B: LN + transposes ----------
            with (
                tc.tile_pool(name="tp_psum", bufs=4, space="PSUM") as tpp,
                tc.tile_pool(name="ld", bufs=3) as ld,
                tc.tile_pool(name="tmp", bufs=3) as tmp,
            ):
                # weights: load natural layout, PE-transpose 128x128 blocks
                for (w_dram, wT, ntile) in (
                    (wq_d, wqT, NOT), (wk_d, wkT, NOT), (wv_d, wvT, NOT),
                ):
                    for i in range(ntile):  # o-tiles of the natural [OH, D]
                        w_nat = ld.tile([P, D], f32, tag="w_nat")
                        nc.sync.dma_start(out=w_nat,
                                          in_=w_dram[i * P:(i + 1) * P, :])
                        for j in range(NFT):
                            ps = tpp.tile([P, P], f32, tag="tp_f32")
                            nc.tensor.transpose(
                                ps, w_nat[:, j * P:(j + 1) * P], ident_f)
                            nc.scalar.copy(
                                out=wT[j][:, i * P:(i + 1) * P], in_=ps)
                # wo: [D, OH] natural -> woT [o, m]
                for mi in range(NFT):
                    w_nat = ld.tile([P, OH], f32, tag="wo_nat")
                    nc.sync.dma_start(out=w_nat,
                                      in_=wo_d[mi * P:(mi + 1) * P, :])
                    for oj in range(NOT):
                        ps = tpp.tile([P, P], f32, tag="tp_f32")
                        nc.tensor.transpose(
                            ps, w_nat[:, oj * P:(oj + 1) * P], ident_f)
                        nc.scalar.copy(
                            out=woT[oj][:, mi * P:(mi + 1) * P], in_=ps)

                # LayerNorm per s-tile, then transpose into xnT
                for st in range(NST):
                    x_t = ld.tile([P, D], f32, tag="x_ln")
                    nc.sync.dma_start(out=x_t, in_=x_d[st * P:(st + 1) * P, :])
                    stats = small.tile([P, 2, 6], f32, tag="stats")
                    for sg in range(2):
                        nc.vector.bn_stats(out=stats[:, sg, :],
                                           in_=x_t[:, sg * 512:(sg + 1) * 512])
                    mv = small.tile([P, 2], f32, tag="mv")
                    nc.vector.bn_aggr(out=mv, in_=stats)
                    rstd = small.tile([P, 1], f32, tag="rstd")
                    nc.scalar.activation(out=rstd, in_=mv[:, 1:2],
                                         func=Act.Sqrt, bias=eps_sb, scale=1.0)
                    nc.vector.reciprocal(out=rstd, in_=rstd)
                    xc = tmp.tile([P, D], f32, tag="xc")
                    nc.vector.tensor_scalar(
                        out=xc, in0=x_t, scalar1=mv[:, 0:1], scalar2=rstd,
                        op0=Alu.subtract, op1=Alu.mult)
                    xn = tmp.tile([P, D], bf16, tag="xn")
                    nc.vector.tensor_mul(out=xn, in0=xc, in1=g_sb)
                    nc.vector.tensor_add(out=xn, in0=xn, in1=b_sb)
                    for j in range(NFT):
                        ps = tpp.tile([P, P], bf16, tag="tp_bf")
                        nc.tensor.transpose(
                            ps, xn[:, j * P:(j + 1) * P], ident_b)
                        nc.scalar.copy(
                            out=xnT[j][:, st * P:(st + 1) * P], in_=ps)

            # ---------- phase C: projections ----------
            with tc.tile_pool(name="proj_psum", bufs=4, space="PSUM") as pp:
                for (wT, dst, bias) in ((wqT, qT, bq_sb), (wkT, kT, bk_sb)):
                    for t in range(NOT):          # output o-tile
                        for c in range(NQS):      # s chunk of 512
                            ps = pp.tile([P, QS], f32, tag="pj")
                            for j in range(NFT):
                                nc.tensor.matmul(
                                    ps,
                                    lhsT=wT[j][:, t * P:(t + 1) * P],
                                    rhs=xnT[j][:, c * QS:(c + 1) * QS],
                                    start=(j == 0), stop=(j == NFT - 1))
                            nc.vector.tensor_scalar_add(
                                out=dst[t][:, c * QS:(c + 1) * QS],
                                in0=ps, scalar1=bias[:, t:t + 1])
                # V: [s, o] layout, with ones columns interleaved per head
                for st in range(NST):
                    nc.gpsimd.memset(vaug[st], 1.0)
                for st in range(NST):
                    ps = pp.tile([P, OH], f32, tag="pj")
                    for j in range(NFT):
                        nc.tensor.matmul(
                            ps,
                            lhsT=xnT[j][:, st * P:(st + 1) * P],
                            rhs=wvT[j],
                            start=(j == 0), stop=(j == NFT - 1))
                    for h in range(HL):
                        nc.vector.tensor_add(
                            out=vaug[st][:, h * (DH + 1):h * (DH + 1) + DH],
                            in0=ps[:, h * DH:(h + 1) * DH],
                            in1=vb_sb[:, h * DH:(h + 1) * DH])

            # ---------- phase D: attention ----------
            with (
                tc.tile_pool(name="s_psum", bufs=3, space="PSUM") as sp,
                tc.tile_pool(name="o_psum", bufs=2, space="PSUM") as op,
                tc.tile_pool(name="pt", bufs=4) as ptp,
                tc.tile_pool(name="nrm", bufs=3) as nrm,
            ):
                for h in range(HL):
                    hq = qT[h // 2][(h % 2) * DH:(h % 2) * DH + DH, :]
                    hk = kT[h // 2][(h % 2) * DH:(h % 2) * DH + DH, :]
                    for qs in range(NQS):
                        nkt = (qs + 1) * KT_PER_QS   # causal: k-tiles 0..nkt-1
                        o_ps = op.tile([DH + 1, QS], f32, tag="o_ps")
                        for kt in range(nkt):
                            s_ps = sp.tile([P, QS], f32, tag="s_ps")
                            nc.tensor.matmul(
                                s_ps,
                                lhsT=hk[:, kt * P:(kt + 1) * P],
                                rhs=hq[:, qs * QS:(qs + 1) * QS],
                                start=True, stop=True, skip_group_check=True)
                            pt = ptp.tile([P, QS], bf16, tag="pt")
                            nc.scalar.activation(
                                out=pt, in_=s_ps, func=Act.Exp,
                                bias=pad_sb[:, kt:kt + 1], scale=0.125)
                            if kt >= qs * KT_PER_QS:  # diagonal region
                                nc.gpsimd.affine_select(
                                    out=pt, in_=pt,
                                    compare_op=Alu.is_ge, fill=0.0,
                                    base=qs * QS - kt * P,
                                    pattern=[[1, QS]], channel_multiplier=-1)
                            nc.tensor.matmul(
                                o_ps,
                                lhsT=vaug[kt][:, h * (DH + 1):(h + 1) * (DH + 1)],
                                rhs=pt,
                                start=(kt == 0), stop=(kt == nkt - 1),
                                skip_group_check=True)
                        dbc = nrm.tile([DH, QS], f32, tag="dbc")
                        nc.vector.reciprocal(out=dbc[0:1, :],
                                             in_=o_ps[DH:DH + 1, :])
                        nc.gpsimd.partition_broadcast(dbc, dbc[0:1, :])
                        nc.vector.tensor_mul(
                            out=oT[h // 2][(h % 2) * DH:(h % 2) * DH + DH,
                                           qs * QS:(qs + 1) * QS],
                            in0=o_ps[0:DH, :], in1=dbc)

            # ---------- phase E: output projection + residual ----------
            with (
                tc.tile_pool(name="y_psum", bufs=4, space="PSUM") as yp,
                tc.tile_pool(name="lde", bufs=3) as lde,
                tc.tile_pool(name="tmpe", bufs=3) as tmpe,
            ):
                for st in range(NST):
                    for mc in range(2):
                        ps = yp.tile([P, QS], f32, tag="y_ps")
                        for ot in range(NOT):
                            nc.tensor.matmul(
                                ps,
                                lhsT=oT[ot][:, st * P:(st + 1) * P],
                                rhs=woT[ot][:, mc * QS:(mc + 1) * QS],
                                start=(ot == 0), stop=(ot == NOT - 1))
                        x_sk = lde.tile([P, QS], f32, tag="x_sk")
                        nc.sync.dma_start(
                            out=x_sk,
                            in_=x_d[st * P:(st + 1) * P, mc * QS:(mc + 1) * QS])
                        y_sb = tmpe.tile([P, QS], f32, tag="y_sb")
                        nc.vector.scalar_tensor_tensor(
                            out=y_sb, in0=x_sk, scalar=0.5, in1=ps,
                            op0=Alu.mult, op1=Alu.add)
                        nc.sync.dma_start(
                            out=out_d[st * P:(st + 1) * P,
                                      mc * QS:(mc + 1) * QS],
                            in_=y_sb)

    nc.compile()
    return nc


def _get_nc():
    if "nc" not in _CACHE:
        _CACHE["nc"] = _build_nc()
    return _CACHE["nc"]


def make_in_maps(x, key_val_lengths, Wq, bq, Wkv, bkv, Wo, bo, ln_g, ln_b):
    x = np.ascontiguousarray(np.asarray(x, dtype=np.float32))
    lens = np.asarray(key_val_lengths).astype(np.int64)
    Wq = np.asarray(Wq, dtype=np.float32)
    Wkv = np.asarray(Wkv, dtype=np.float32)
    Wo = np.asarray(Wo, dtype=np.float32)
    bq = np.asarray(bq, dtype=np.float32)
    bkv = np.asarray(bkv, dtype=np.float32)
    ln_g = np.asarray(ln_g, dtype=np.float32)
    ln_b = np.asarray(ln_b, dtype=np.float32)

    in_maps = []
    for core in range(8):
        b, half = divmod(core, 2)
        sl = slice(half * OH, (half + 1) * OH)
        pad = np.where(np.arange(S) < lens[b], 0.0, NEG).astype(np.float32)
        in_maps.append({
            "x": x[b],
            "wq": np.ascontiguousarray(Wq[sl]),
            "wk": np.ascontiguousarray(Wkv[sl]),
            "wv": np.ascontiguousarray(Wkv[D + half * OH:D + (half + 1) * OH]),
            "wo": np.ascontiguousarray(Wo[:, sl]),
            "bq": np.ascontiguousarray(bq[sl]),
            "bk": np.ascontiguousarray(bkv[sl]),
            "bv": np.ascontiguousarray(bkv[D + half * OH:D + (half + 1) * OH]),
            "ln_g": ln_g,
            "ln_b": ln_b,
            "pad": np.ascontiguousarray(pad.reshape(NST, P).T),
        })
    return in_maps


def kernel(**inputs):
    from concourse.bass_utils import run_bass_kernel_spmd

    nc = _get_nc()
    in_maps = make_in_maps(**inputs)
    res = run_bass_kernel_spmd(
        nc, in_maps, core_ids=list(range(8)),
        trace=bool(os.environ.get("KERNEL_TRACE")))
    _CACHE["last_results"] = res
    bo = np.asarray(inputs["bo"], dtype=np.float32)
    y = np.empty((B, S, D), dtype=np.float32)
    for b in range(B):
        y[b] = res.results[2 * b]["out"] + res.results[2 * b + 1]["out"] + bo
    return y


# revision 16
# speedup vs baseline: 1.0937x; 1.0937x over previous
"""Trainium2 Bass kernel: causal multi-head self-attention block (pre-LN).

Full module computed on 8 NeuronCores:
    xn = LayerNorm(x); q = xn@Wq.T+bq; k,v = xn@Wkv.T+bkv
    out = softmax(mask(q k^T / sqrt(dh))) v @ Wo.T + bo + x

Sharding: core = batch_index * 2 + head_half.  Each core handles one batch
element and 8 of the 16 heads (column-parallel QKV, row-parallel Wo), emits a
partial [S, D] output including half the residual; host sums core pairs and
adds bo.  Weights are pre-transposed and cast to bf16 on the host so they DMA
straight into the matmul-ready layout.

Shapes are hardcoded for B=4, S=2048, D=1024, H=16, DH=64.
"""

import os
import sys

import numpy as np

sys.path.insert(0, "/opt/trn_rl_repo")

B, S, D, H = 4, 2048, 1024, 16
DH = D // H            # 64
HL = H // 2            # heads per core: 8
OH = HL * DH           # per-core head features: 512
EPS = 1e-5
NEG = -30000.0         # additive mask; exp(x + NEG) underflows to 0
P = 128                # SBUF partitions
NST = S // P           # 16 s-tiles
NFT = D // P           # 8 feature tiles
NOT = OH // P          # 4 o-tiles (per-core head features)
QS = 512               # query super-tile (matmul moving free dim)
NQS = S // QS          # 4
KT_PER_QS = QS // P    # 4 k-tiles per q-super

_CACHE = {}


def _build_nc():
    import concourse.bass as bass
    import concourse.bacc as bacc
    import concourse.tile as tile
    from concourse import mybir

    f32 = mybir.dt.float32
    bf16 = mybir.dt.bfloat16
    Alu = mybir.AluOpType
    Act = mybir.ActivationFunctionType

    nc = bacc.Bacc("TRN2", target_bir_lowering=False, debug=False, num_devices=8)

    # ---- DRAM I/O (per-core shard shapes; w* pre-transposed + bf16 on host) ----
    x_d = nc.dram_tensor("x", [S, D], f32, kind="ExternalInput").ap()
    wq_d = nc.dram_tensor("wqt", [D, OH], bf16, kind="ExternalInput").ap()
    wk_d = nc.dram_tensor("wkt", [D, OH], bf16, kind="ExternalInput").ap()
    wv_d = nc.dram_tensor("wvt", [D, OH], bf16, kind="ExternalInput").ap()
    wo_d = nc.dram_tensor("wot", [OH, D], bf16, kind="ExternalInput").ap()
    bq_d = nc.dram_tensor("bq", [OH], f32, kind="ExternalInput").ap()
    bk_d = nc.dram_tensor("bk", [OH], f32, kind="ExternalInput").ap()
    bv_d = nc.dram_tensor("bv", [OH], f32, kind="ExternalInput").ap()
    g_d = nc.dram_tensor("ln_g", [D], f32, kind="ExternalInput").ap()
    b_d = nc.dram_tensor("ln_b", [D], f32, kind="ExternalInput").ap()
    pad_d = nc.dram_tensor("pad", [P, NST], f32, kind="ExternalInput").ap()
    out_d = nc.dram_tensor("out", [S, D], f32, kind="ExternalOutput").ap()
    debug = bool(os.environ.get("KERNEL_DEBUG"))
    if debug:
        dbg = {n: nc.dram_tensor(f"dbg_{n}", shp, bf16, kind="ExternalOutput").ap()
               for n, shp in (("xnT0", [P, S]), ("qT0", [P, S]), ("kT0", [P, S]),
                              ("vaug0", [P, HL * (DH + 1)]), ("oT0", [P, S]),
                              ("wqT0", [P, OH]))}

    def bcast(ap_1d, n):
        # [n] dram vector -> [P, n] partition-broadcast DMA source
        return bass.AP(tensor=ap_1d.tensor, offset=ap_1d.offset,
                       ap=[[0, P], [1, n]])

    with tile.TileContext(nc) as tc:
        with (
            tc.tile_pool(name="res", bufs=1) as res,       # resident tensors
            tc.tile_pool(name="small", bufs=4) as small,
        ):
            # ---------- constants ----------
            g_sb = res.tile([P, D], f32, tag="g_sb")
            nc.sync.dma_start(out=g_sb, in_=bcast(g_d, D))
            b_sb = res.tile([P, D], f32, tag="b_sb")
            nc.sync.dma_start(out=b_sb, in_=bcast(b_d, D))
            vb_sb = res.tile([P, OH], f32, tag="vb_sb")
            nc.sync.dma_start(out=vb_sb, in_=bcast(bv_d, OH))
            pad_sb = res.tile([P, NST], f32, tag="pad_sb")
            nc.sync.dma_start(out=pad_sb, in_=pad_d)
            bq_sb = res.tile([P, NOT], f32, tag="bq_sb")
            nc.sync.dma_start(out=bq_sb, in_=bq_d.rearrange("(t p) -> p t", p=P))
            bk_sb = res.tile([P, NOT], f32, tag="bk_sb")
            nc.sync.dma_start(out=bk_sb, in_=bk_d.rearrange("(t p) -> p t", p=P))
            eps_sb = res.tile([P, 1], f32, tag="eps_sb")
            nc.vector.memset(eps_sb, EPS)
            ident_b = res.tile([P, P], bf16, tag="ident_b")
            nc.gpsimd.memset(ident_b, 0.0)
            nc.gpsimd.affine_select(
                out=ident_b, in_=ident_b, compare_op=Alu.not_equal, fill=1.0,
                base=0, pattern=[[-1, P]], channel_multiplier=1)
            # causal masks for the 4 diagonal offsets: keep q_l - k_p >= -128*i
            tri = []
            for i in range(KT_PER_QS):
                t_i = res.tile([P, QS], bf16, tag=f"tri{i}", name=f"tri{i}")
                nc.gpsimd.memset(t_i, 1.0)
                nc.gpsimd.affine_select(
                    out=t_i, in_=t_i, compare_op=Alu.is_ge, fill=0.0,
                    base=-P * i, pattern=[[1, QS]], channel_multiplier=-1)
                tri.append(t_i)

            # ---------- resident big tensors ----------
            xnT = [res.tile([P, S], bf16, tag=f"xnT{j}", name=f"xnT{j}")
                   for j in range(NFT)]
            qT = [res.tile([P, S], bf16, tag=f"qT{t}", name=f"qT{t}")
                  for t in range(NOT)]
            kT = [res.tile([P, S], bf16, tag=f"kT{t}", name=f"kT{t}")
                  for t in range(NOT)]
            # V augmented with a ones column per head: [s, h*65 .. h*65+64]
            vaug = [res.tile([P, HL * (DH + 1)], bf16, tag=f"vaug{i}",
                             name=f"vaug{i}") for i in range(NST)]
            oT = [res.tile([P, S], bf16, tag=f"oT{t}", name=f"oT{t}")
                  for t in range(NOT)]
            wqT = [res.tile([P, OH], bf16, tag=f"wqT{j}", name=f"wqT{j}")
                   for j in range(NFT)]
            wkT = [res.tile([P, OH], bf16, tag=f"wkT{j}", name=f"wkT{j}")
                   for j in range(NFT)]
            wvT = [res.tile([P, OH], bf16, tag=f"wvT{j}", name=f"wvT{j}")
                   for j in range(NFT)]
            woT = [res.tile([P, D], bf16, tag=f"woT{t}", name=f"woT{t}")
                   for t in range(NOT)]

            # weights DMA straight into matmul layout (host pre-transposed)
            for j in range(NFT):
                nc.sync.dma_start(out=wqT[j], in_=wq_d[j * P:(j + 1) * P, :])
                nc.sync.dma_start(out=wkT[j], in_=wk_d[j * P:(j + 1) * P, :])
                nc.sync.dma_start(out=wvT[j], in_=wv_d[j * P:(j + 1) * P, :])
            for t in range(NOT):
                nc.sync.dma_start(out=woT[t], in_=wo_d[t * P:(t + 1) * P, :])

            # ---------- phase A: LayerNorm + transpose into xnT ----------
            with (
                tc.tile_pool(name="ld", bufs=3) as ld,
                tc.tile_pool(name="tmp", bufs=3) as tmp,
                tc.tile_pool(name="tp_psum", bufs=4, space="PSUM") as tpp,
            ):
                for st in range(NST):
                    x_t = ld.tile([P, D], f32, tag="x_ln")
                    nc.sync.dma_start(out=x_t, in_=x_d[st * P:(st + 1) * P, :])
                    stats = small.tile([P, 2, 6], f32, tag="stats")
                    for sg in range(2):
                        nc.vector.bn_stats(out=stats[:, sg, :],
                                           in_=x_t[:, sg * 512:(sg + 1) * 512])
                    mv = small.tile([P, 2], f32, tag="mv")
                    nc.vector.bn_aggr(out=mv, in_=stats)
                    rstd = small.tile([P, 1], f32, tag="rstd")
                    nc.scalar.activation(out=rstd, in_=mv[:, 1:2],
                                         func=Act.Sqrt, bias=eps_sb, scale=1.0)
                    nc.vector.reciprocal(out=rstd, in_=rstd)
                    xc = tmp.tile([P, D], f32, tag="xc")
                    nc.vector.tensor_scalar(
                        out=xc, in0=x_t, scalar1=mv[:, 0:1], scalar2=rstd,
                        op0=Alu.subtract, op1=Alu.mult)
                    xn = tmp.tile([P, D], bf16, tag="xn")
                    nc.vector.tensor_mul(out=xn, in0=xc, in1=g_sb)
                    nc.vector.tensor_add(out=xn, in0=xn, in1=b_sb)
                    for j in range(NFT):
                        ps = tpp.tile([P, P], bf16, tag="tp_bf")
                        nc.tensor.transpose(
                            ps, xn[:, j * P:(j + 1) * P], ident_b)
                        nc.scalar.copy(
                            out=xnT[j][:, st * P:(st + 1) * P], in_=ps)

            # ---------- phase C: projections ----------
            with tc.tile_pool(name="proj_psum", bufs=4, space="PSUM") as pp:
                for (wT, dst, bias) in ((wqT, qT, bq_sb), (wkT, kT, bk_sb)):
                    for t in range(NOT):          # output o-tile
                        for c in range(NQS):      # s chunk of 512
                            ps = pp.tile([P, QS], f32, tag="pj")
                            for j in range(NFT):
                                nc.tensor.matmul(
                                    ps,
                                    lhsT=wT[j][:, t * P:(t + 1) * P],
                                    rhs=xnT[j][:, c * QS:(c + 1) * QS],
                                    start=(j == 0), stop=(j == NFT - 1))
                            nc.vector.tensor_scalar_add(
                                out=dst[t][:, c * QS:(c + 1) * QS],
                                in0=ps, scalar1=bias[:, t:t + 1])
                # V: [s, o] layout, with ones columns interleaved per head
                for st in range(NST):
                    nc.gpsimd.memset(vaug[st], 1.0)
                for st in range(NST):
                    ps = pp.tile([P, OH], f32, tag="pj")
                    for j in range(NFT):
                        nc.tensor.matmul(
                            ps,
                            lhsT=xnT[j][:, st * P:(st + 1) * P],
                            rhs=wvT[j],
                            start=(j == 0), stop=(j == NFT - 1))
                    for h in range(HL):
                        nc.vector.tensor_add(
                            out=vaug[st][:, h * (DH + 1):h * (DH + 1) + DH],
                            in0=ps[:, h * DH:(h + 1) * DH],
                            in1=vb_sb[:, h * DH:(h + 1) * DH])

            # ---------- phase D: attention ----------
            with (
                tc.tile_pool(name="s_psum", bufs=4, space="PSUM") as sp,
                tc.tile_pool(name="o_psum", bufs=2, space="PSUM") as op,
                tc.tile_pool(name="pt", bufs=6) as ptp,
                tc.tile_pool(name="nrm", bufs=3) as nrm,
            ):
                for h in range(HL):
                    hq = qT[h // 2][(h % 2) * DH:(h % 2) * DH + DH, :]
                    hk = kT[h // 2][(h % 2) * DH:(h % 2) * DH + DH, :]
                    for qs in range(NQS):
                        nkt = (qs + 1) * KT_PER_QS   # causal: k-tiles 0..nkt-1
                        o_ps = op.tile([DH + 1, QS], f32, tag="o_ps")
                        for kt in range(nkt):
                            s_ps = sp.tile([P, QS], f32, tag="s_ps")
                            nc.tensor.matmul(
                                s_ps,
                                lhsT=hk[:, kt * P:(kt + 1) * P],
                                rhs=hq[:, qs * QS:(qs + 1) * QS],
                                start=True, stop=True, skip_group_check=True)
                            pt = ptp.tile([P, QS], bf16, tag="pt")
                            nc.scalar.activation(
                                out=pt, in_=s_ps, func=Act.Exp,
                                bias=pad_sb[:, kt:kt + 1], scale=0.125)
                            if kt >= qs * KT_PER_QS:  # diagonal region
                                nc.vector.tensor_mul(
                                    out=pt, in0=pt,
                                    in1=tri[kt - qs * KT_PER_QS])
                            nc.tensor.matmul(
                                o_ps,
                                lhsT=vaug[kt][:, h * (DH + 1):(h + 1) * (DH + 1)],
                                rhs=pt,
                                start=(kt == 0), stop=(kt == nkt - 1),
                                skip_group_check=True)
                        den_sb = nrm.tile([1, QS], f32, tag="den_sb")
                        nc.vector.tensor_copy(den_sb, o_ps[DH:DH + 1, :])
                        dbc = nrm.tile([DH, QS], f32, tag="dbc")
                        nc.vector.reciprocal_approx_fast(
                            out=dbc[0:1, :], in_=den_sb)
                        nc.gpsimd.partition_broadcast(dbc, dbc[0:1, :])
                        nc.vector.tensor_mul(
                            out=oT[h // 2][(h % 2) * DH:(h % 2) * DH + DH,
                                           qs * QS:(qs + 1) * QS],
                            in0=o_ps[0:DH, :], in1=dbc)

            # ---------- phase E: output projection + residual ----------
            with (
                tc.tile_pool(name="y_psum", bufs=4, space="PSUM") as yp,
                tc.tile_pool(name="lde", bufs=3) as lde,
                tc.tile_pool(name="tmpe", bufs=3) as tmpe,
            ):
                for st in range(NST):
                    for mc in range(2):
                        ps = yp.tile([P, QS], f32, tag="y_ps")
                        for ot in range(NOT):
                            nc.tensor.matmul(
                                ps,
                                lhsT=oT[ot][:, st * P:(st + 1) * P],
                                rhs=woT[ot][:, mc * QS:(mc + 1) * QS],
                                start=(ot == 0), stop=(ot == NOT - 1))
                        x_sk = lde.tile([P, QS], f32, tag="x_sk")
                        nc.sync.dma_start(
                            out=x_sk,
                            in_=x_d[st * P:(st + 1) * P, mc * QS:(mc + 1) * QS])
                        y_sb = tmpe.tile([P, QS], f32, tag="y_sb")
                        nc.vector.scalar_tensor_tensor(
                            out=y_sb, in0=x_sk, scalar=0.5, in1=ps,
                            op0=Alu.mult, op1=Alu.add)
                        nc.sync.dma_start(
                            out=out_d[st * P:(st + 1) * P,
                                      mc * QS:(mc + 1) * QS],
                            in_=y_sb)

            if debug:
                nc.sync.dma_start(out=dbg["xnT0"], in_=xnT[0])
                nc.sync.dma_start(out=dbg["qT0"], in_=qT[0])
                nc.sync.dma_start(out=dbg["kT0"], in_=kT[0])
                nc.sync.dma_start(out=dbg["vaug0"], in_=vaug[0])
                nc.sync.dma_start(out=dbg["oT0"], in_=oT[0])
                nc.sync.dma_start(out=dbg["wqT0"], in_=wqT[0])

    nc.compile()
    return nc


def _get_nc():
    if "nc" not in _CACHE:
        _CACHE["nc"] = _build_nc()
    return _CACHE["nc"]


def make_in_maps(x, key_val_lengths, Wq, bq, Wkv, bkv, Wo, bo, ln_g, ln_b):
    import ml_dtypes
    bf = ml_dtypes.bfloat16

    x = np.ascontiguousarray(np.asarray(x, dtype=np.float32))
    lens = np.asarray(key_val_lengths).astype(np.int64)
    Wq = np.asarray(Wq, dtype=np.float32)
    Wkv = np.asarray(Wkv, dtype=np.float32)
    Wo = np.asarray(Wo, dtype=np.float32)
    bq = np.asarray(bq, dtype=np.float32)
    bkv = np.asarray(bkv, dtype=np.float32)
    ln_g = np.asarray(ln_g, dtype=np.float32)
    ln_b = np.asarray(ln_b, dtype=np.float32)

    in_maps = []
    for core in range(8):
        b, half = divmod(core, 2)
        sl = slice(half * OH, (half + 1) * OH)
        vsl = slice(D + half * OH, D + (half + 1) * OH)
        pad = np.where(np.arange(S) < lens[b], 0.0, NEG).astype(np.float32)
        in_maps.append({
            "x": x[b],
            "wqt": np.ascontiguousarray(Wq[sl].T.astype(bf)),
            "wkt": np.ascontiguousarray(Wkv[sl].T.astype(bf)),
            "wvt": np.ascontiguousarray(Wkv[vsl].T.astype(bf)),
            "wot": np.ascontiguousarray(Wo[:, sl].T.astype(bf)),
            "bq": np.ascontiguousarray(bq[sl]),
            "bk": np.ascontiguousarray(bkv[sl]),
            "bv": np.ascontiguousarray(bkv[vsl]),
            "ln_g": ln_g,
            "ln_b": ln_b,
            "pad": np.ascontiguousarray(pad.reshape(NST, P).T),
        })
    return in_maps


def kernel(**inputs):
    from concourse.bass_utils import run_bass_kernel_spmd

    nc = _get_nc()
    in_maps = make_in_maps(**inputs)
    res = run_bass_kernel_spmd(
        nc, in_maps, core_ids=list(range(8)),
        trace=bool(os.environ.get("KERNEL_TRACE")))
    _CACHE["last_results"] = res
    bo = np.asarray(inputs["bo"], dtype=np.float32)
    y = np.empty((B, S, D), dtype=np.float32)
    for b in range(B):
        y[b] = res.results[2 * b]["out"] + res.results[2 * b + 1]["out"] + bo
    return y


# revision 17
# speedup vs baseline: 1.2196x; 1.1152x over previous
"""Trainium2 Bass kernel: causal multi-head self-attention block (pre-LN).

Full module computed on 8 NeuronCores:
    xn = LayerNorm(x); q = xn@Wq.T+bq; k,v = xn@Wkv.T+bkv
    out = softmax(mask(q k^T / sqrt(dh))) v @ Wo.T + bo + x

Sharding: core = batch_index * 2 + head_half.  Each core handles one batch
element and 8 of the 16 heads (column-parallel QKV, row-parallel Wo), emits a
partial [S, D] output including half the residual; host sums core pairs and
adds bo.  Weights are pre-transposed and cast to bf16 on the host so they DMA
straight into the matmul-ready layout.

Shapes are hardcoded for B=4, S=2048, D=1024, H=16, DH=64.
"""

import os
import sys

import numpy as np

sys.path.insert(0, "/opt/trn_rl_repo")

B, S, D, H = 4, 2048, 1024, 16
DH = D // H            # 64
HL = H // 2            # heads per core: 8
OH = HL * DH           # per-core head features: 512
EPS = 1e-5
NEG = -30000.0         # additive mask; exp(x + NEG) underflows to 0
P = 128                # SBUF partitions
NST = S // P           # 16 s-tiles
NFT = D // P           # 8 feature tiles
NOT = OH // P          # 4 o-tiles (per-core head features)
QS = 512               # query super-tile (matmul moving free dim)
NQS = S // QS          # 4
KT_PER_QS = QS // P    # 4 k-tiles per q-super

_CACHE = {}


def _build_nc():
    import concourse.bass as bass
    import concourse.bacc as bacc
    import concourse.tile as tile
    from concourse import mybir

    f32 = mybir.dt.float32
    bf16 = mybir.dt.bfloat16
    Alu = mybir.AluOpType
    Act = mybir.ActivationFunctionType

    nc = bacc.Bacc("TRN2", target_bir_lowering=False, debug=False, num_devices=8)

    # ---- DRAM I/O (per-core shard shapes; w* pre-transposed + bf16 on host) ----
    x_d = nc.dram_tensor("x", [S, D], f32, kind="ExternalInput").ap()
    wq_d = nc.dram_tensor("wqt", [D, OH], bf16, kind="ExternalInput").ap()
    wk_d = nc.dram_tensor("wkt", [D, OH], bf16, kind="ExternalInput").ap()
    wv_d = nc.dram_tensor("wvt", [D, OH], bf16, kind="ExternalInput").ap()
    wo_d = nc.dram_tensor("wot", [OH, D], bf16, kind="ExternalInput").ap()
    bq_d = nc.dram_tensor("bq", [OH], f32, kind="ExternalInput").ap()
    bk_d = nc.dram_tensor("bk", [OH], f32, kind="ExternalInput").ap()
    bv_d = nc.dram_tensor("bv", [OH], f32, kind="ExternalInput").ap()
    g_d = nc.dram_tensor("ln_g", [D], f32, kind="ExternalInput").ap()
    b_d = nc.dram_tensor("ln_b", [D], f32, kind="ExternalInput").ap()
    pad_d = nc.dram_tensor("pad", [P, NST], f32, kind="ExternalInput").ap()
    out_d = nc.dram_tensor("out", [S, D], f32, kind="ExternalOutput").ap()
    debug = bool(os.environ.get("KERNEL_DEBUG"))
    if debug:
        dbg = {n: nc.dram_tensor(f"dbg_{n}", shp, bf16, kind="ExternalOutput").ap()
               for n, shp in (("xnT0", [P, S]), ("qT0", [P, S]), ("kT0", [P, S]),
                              ("vaug0", [P, HL * (DH + 1)]), ("oT0", [P, S]),
                              ("wqT0", [P, OH]))}

    def bcast(ap_1d, n):
        # [n] dram vector -> [P, n] partition-broadcast DMA source
        return bass.AP(tensor=ap_1d.tensor, offset=ap_1d.offset,
                       ap=[[0, P], [1, n]])

    with tile.TileContext(nc) as tc:
        with (
            tc.tile_pool(name="res", bufs=1) as res,       # resident tensors
            tc.tile_pool(name="small", bufs=4) as small,
        ):
            # ---------- constants ----------
            g_sb = res.tile([P, D], f32, tag="g_sb")
            nc.sync.dma_start(out=g_sb, in_=bcast(g_d, D))
            b_sb = res.tile([P, D], f32, tag="b_sb")
            nc.sync.dma_start(out=b_sb, in_=bcast(b_d, D))
            vb_sb = res.tile([P, OH], f32, tag="vb_sb")
            nc.sync.dma_start(out=vb_sb, in_=bcast(bv_d, OH))
            pad_sb = res.tile([P, NST], f32, tag="pad_sb")
            nc.sync.dma_start(out=pad_sb, in_=pad_d)
            bq_sb = res.tile([P, NOT], f32, tag="bq_sb")
            nc.sync.dma_start(out=bq_sb, in_=bq_d.rearrange("(t p) -> p t", p=P))
            bk_sb = res.tile([P, NOT], f32, tag="bk_sb")
            nc.sync.dma_start(out=bk_sb, in_=bk_d.rearrange("(t p) -> p t", p=P))
            eps_sb = res.tile([P, 1], f32, tag="eps_sb")
            nc.vector.memset(eps_sb, EPS)
            ident_b = res.tile([P, P], bf16, tag="ident_b")
            nc.gpsimd.memset(ident_b, 0.0)
            nc.gpsimd.affine_select(
                out=ident_b, in_=ident_b, compare_op=Alu.not_equal, fill=1.0,
                base=0, pattern=[[-1, P]], channel_multiplier=1)
            # causal masks for the 4 diagonal offsets: keep q_l - k_p >= -128*i
            tri = []
            for i in range(KT_PER_QS):
                t_i = res.tile([P, QS], bf16, tag=f"tri{i}", name=f"tri{i}")
                nc.gpsimd.memset(t_i, 1.0)
                nc.gpsimd.affine_select(
                    out=t_i, in_=t_i, compare_op=Alu.is_ge, fill=0.0,
                    base=-P * i, pattern=[[1, QS]], channel_multiplier=-1)
                tri.append(t_i)

            # ---------- resident big tensors ----------
            xnT = [res.tile([P, S], bf16, tag=f"xnT{j}", name=f"xnT{j}")
                   for j in range(NFT)]
            qT = [res.tile([P, S], bf16, tag=f"qT{t}", name=f"qT{t}")
                  for t in range(NOT)]
            kT = [res.tile([P, S], bf16, tag=f"kT{t}", name=f"kT{t}")
                  for t in range(NOT)]
            # V augmented with a ones column per head: [s, h*65 .. h*65+64]
            vaug = [res.tile([P, HL * (DH + 1)], bf16, tag=f"vaug{i}",
                             name=f"vaug{i}") for i in range(NST)]
            oT = [res.tile([P, S], bf16, tag=f"oT{t}", name=f"oT{t}")
                  for t in range(NOT)]
            wqT = [res.tile([P, OH], bf16, tag=f"wqT{j}", name=f"wqT{j}")
                   for j in range(NFT)]
            wkT = [res.tile([P, OH], bf16, tag=f"wkT{j}", name=f"wkT{j}")
                   for j in range(NFT)]
            wvT = [res.tile([P, OH], bf16, tag=f"wvT{j}", name=f"wvT{j}")
                   for j in range(NFT)]
            woT = [res.tile([P, D], bf16, tag=f"woT{t}", name=f"woT{t}")
                   for t in range(NOT)]

            # weights DMA straight into matmul layout (host pre-transposed)
            for j in range(NFT):
                nc.sync.dma_start(out=wqT[j], in_=wq_d[j * P:(j + 1) * P, :])
                nc.sync.dma_start(out=wkT[j], in_=wk_d[j * P:(j + 1) * P, :])
                nc.sync.dma_start(out=wvT[j], in_=wv_d[j * P:(j + 1) * P, :])
            for t in range(NOT):
                nc.sync.dma_start(out=woT[t], in_=wo_d[t * P:(t + 1) * P, :])

            # ---------- phase A: LayerNorm + transpose into xnT ----------
            with (
                tc.tile_pool(name="ld", bufs=3) as ld,
                tc.tile_pool(name="tmp", bufs=3) as tmp,
                tc.tile_pool(name="tp_psum", bufs=4, space="PSUM") as tpp,
            ):
                for st in range(NST):
                    x_t = ld.tile([P, D], f32, tag="x_ln")
                    nc.sync.dma_start(out=x_t, in_=x_d[st * P:(st + 1) * P, :])
                    stats = small.tile([P, 2, 6], f32, tag="stats")
                    for sg in range(2):
                        nc.vector.bn_stats(out=stats[:, sg, :],
                                           in_=x_t[:, sg * 512:(sg + 1) * 512])
                    mv = small.tile([P, 2], f32, tag="mv")
                    nc.vector.bn_aggr(out=mv, in_=stats)
                    rstd = small.tile([P, 1], f32, tag="rstd")
                    nc.scalar.activation(out=rstd, in_=mv[:, 1:2],
                                         func=Act.Sqrt, bias=eps_sb, scale=1.0)
                    nc.vector.reciprocal(out=rstd, in_=rstd)
                    xc = tmp.tile([P, D], f32, tag="xc")
                    nc.vector.tensor_scalar(
                        out=xc, in0=x_t, scalar1=mv[:, 0:1], scalar2=rstd,
                        op0=Alu.subtract, op1=Alu.mult)
                    xn = tmp.tile([P, D], bf16, tag="xn")
                    nc.vector.tensor_mul(out=xn, in0=xc, in1=g_sb)
                    nc.vector.tensor_add(out=xn, in0=xn, in1=b_sb)
                    for j in range(NFT):
                        ps = tpp.tile([P, P], bf16, tag="tp_bf")
                        nc.tensor.transpose(
                            ps, xn[:, j * P:(j + 1) * P], ident_b)
                        nc.scalar.copy(
                            out=xnT[j][:, st * P:(st + 1) * P], in_=ps)

            # ---------- phases C/D/E interleaved ----------
            # s-chunk-major projections, then per-q-super attention + output
            # projection, so PE always has dense independent work in flight.
            with (
                tc.tile_pool(name="pj_psum", bufs=3, space="PSUM") as pp,
                tc.tile_pool(name="s_psum", bufs=3, space="PSUM") as sp,
                tc.tile_pool(name="o_psum", bufs=2, space="PSUM") as op,
                tc.tile_pool(name="pt", bufs=6) as ptp,
                tc.tile_pool(name="nrm", bufs=3) as nrm,
                tc.tile_pool(name="lde", bufs=3) as lde,
                tc.tile_pool(name="tmpe", bufs=3) as tmpe,
            ):
                for st in range(NST):
                    nc.gpsimd.memset(vaug[st], 1.0)
                for c in range(NQS):
                    # qT / kT chunks for s-range [c*512, (c+1)*512)
                    for (wT, dst, bias) in ((wqT, qT, bq_sb), (wkT, kT, bk_sb)):
                        for t in range(NOT):
                            ps = pp.tile([P, QS], f32, tag="pj")
                            for j in range(NFT):
                                nc.tensor.matmul(
                                    ps,
                                    lhsT=wT[j][:, t * P:(t + 1) * P],
                                    rhs=xnT[j][:, c * QS:(c + 1) * QS],
                                    start=(j == 0), stop=(j == NFT - 1))
                            nc.vector.tensor_scalar_add(
                                out=dst[t][:, c * QS:(c + 1) * QS],
                                in0=ps, scalar1=bias[:, t:t + 1])
                    # V tiles for the same s-range
                    for st in range(c * KT_PER_QS, (c + 1) * KT_PER_QS):
                        ps = pp.tile([P, OH], f32, tag="pj")
                        for j in range(NFT):
                            nc.tensor.matmul(
                                ps,
                                lhsT=xnT[j][:, st * P:(st + 1) * P],
                                rhs=wvT[j],
                                start=(j == 0), stop=(j == NFT - 1))
                        for h in range(HL):
                            nc.vector.tensor_add(
                                out=vaug[st][:, h * (DH + 1):h * (DH + 1) + DH],
                                in0=ps[:, h * DH:(h + 1) * DH],
                                in1=vb_sb[:, h * DH:(h + 1) * DH])

                for qs in range(NQS):
                    nkt = (qs + 1) * KT_PER_QS   # causal: k-tiles 0..nkt-1
                    for h in range(HL):
                        hq = qT[h // 2][(h % 2) * DH:(h % 2) * DH + DH, :]
                        hk = kT[h // 2][(h % 2) * DH:(h % 2) * DH + DH, :]
                        o_ps = op.tile([DH + 1, QS], f32, tag="o_ps")
                        for kt in range(nkt):
                            s_ps = sp.tile([P, QS], f32, tag="s_ps")
                            nc.tensor.matmul(
                                s_ps,
                                lhsT=hk[:, kt * P:(kt + 1) * P],
                                rhs=hq[:, qs * QS:(qs + 1) * QS],
                                start=True, stop=True, skip_group_check=True)
                            pt = ptp.tile([P, QS], bf16, tag="pt")
                            nc.scalar.activation(
                                out=pt, in_=s_ps, func=Act.Exp,
                                bias=pad_sb[:, kt:kt + 1], scale=0.125)
                            if kt >= qs * KT_PER_QS:  # diagonal region
                                nc.vector.tensor_mul(
                                    out=pt, in0=pt,
                                    in1=tri[kt - qs * KT_PER_QS])
                            nc.tensor.matmul(
                                o_ps,
                                lhsT=vaug[kt][:, h * (DH + 1):(h + 1) * (DH + 1)],
                                rhs=pt,
                                start=(kt == 0), stop=(kt == nkt - 1),
                                skip_group_check=True)
                        den_sb = nrm.tile([1, QS], f32, tag="den_sb")
                        nc.vector.tensor_copy(den_sb, o_ps[DH:DH + 1, :])
                        dbc = nrm.tile([DH, QS], f32, tag="dbc")
                        nc.vector.reciprocal_approx_fast(
                            out=dbc[0:1, :], in_=den_sb)
                        nc.gpsimd.partition_broadcast(dbc, dbc[0:1, :])
                        nc.vector.tensor_mul(
                            out=oT[h // 2][(h % 2) * DH:(h % 2) * DH + DH,
                                           qs * QS:(qs + 1) * QS],
                            in0=o_ps[0:DH, :], in1=dbc)

                    # output projection for the s-tiles this q-super covers
                    for st in range(qs * KT_PER_QS, (qs + 1) * KT_PER_QS):
                        for mc in range(2):
                            ps = pp.tile([P, QS], f32, tag="pj")
                            for ot in range(NOT):
                                nc.tensor.matmul(
                                    ps,
                                    lhsT=oT[ot][:, st * P:(st + 1) * P],
                                    rhs=woT[ot][:, mc * QS:(mc + 1) * QS],
                                    start=(ot == 0), stop=(ot == NOT - 1))
                            x_sk = lde.tile([P, QS], f32, tag="x_sk")
                            nc.sync.dma_start(
                                out=x_sk,
                                in_=x_d[st * P:(st + 1) * P,
                                        mc * QS:(mc + 1) * QS])
                            y_sb = tmpe.tile([P, QS], f32, tag="y_sb")
                            nc.vector.scalar_tensor_tensor(
                                out=y_sb, in0=x_sk, scalar=0.5, in1=ps,
                                op0=Alu.mult, op1=Alu.add)
                            nc.sync.dma_start(
                                out=out_d[st * P:(st + 1) * P,
                                          mc * QS:(mc + 1) * QS],
                                in_=y_sb)

    nc.compile()
    return nc


def _get_nc():
    if "nc" not in _CACHE:
        _CACHE["nc"] = _build_nc()
    return _CACHE["nc"]


def make_in_maps(x, key_val_lengths, Wq, bq, Wkv, bkv, Wo, bo, ln_g, ln_b):
    import ml_dtypes
    bf = ml_dtypes.bfloat16

    x = np.ascontiguousarray(np.asarray(x, dtype=np.float32))
    lens = np.asarray(key_val_lengths).astype(np.int64)
    Wq = np.asarray(Wq, dtype=np.float32)
    Wkv = np.asarray(Wkv, dtype=np.float32)
    Wo = np.asarray(Wo, dtype=np.float32)
    bq = np.asarray(bq, dtype=np.float32)
    bkv = np.asarray(bkv, dtype=np.float32)
    ln_g = np.asarray(ln_g, dtype=np.float32)
    ln_b = np.asarray(ln_b, dtype=np.float32)

    in_maps = []
    for core in range(8):
        b, half = divmod(core, 2)
        sl = slice(half * OH, (half + 1) * OH)
        vsl = slice(D + half * OH, D + (half + 1) * OH)
        pad = np.where(np.arange(S) < lens[b], 0.0, NEG).astype(np.float32)
        in_maps.append({
            "x": x[b],
            "wqt": np.ascontiguousarray(Wq[sl].T.astype(bf)),
            "wkt": np.ascontiguousarray(Wkv[sl].T.astype(bf)),
            "wvt": np.ascontiguousarray(Wkv[vsl].T.astype(bf)),
            "wot": np.ascontiguousarray(Wo[:, sl].T.astype(bf)),
            "bq": np.ascontiguousarray(bq[sl]),
            "bk": np.ascontiguousarray(bkv[sl]),
            "bv": np.ascontiguousarray(bkv[vsl]),
            "ln_g": ln_g,
            "ln_b": ln_b,
            "pad": np.ascontiguousarray(pad.reshape(NST, P).T),
        })
    return in_maps


def kernel(**inputs):
    from concourse.bass_utils import run_bass_kernel_spmd

    nc = _get_nc()
    in_maps = make_in_maps(**inputs)
    res = run_bass_kernel_spmd(
        nc, in_maps, core_ids=list(range(8)),
        trace=bool(os.environ.get("KERNEL_TRACE")))
    _CACHE["last_results"] = res
    bo = np.asarray(inputs["bo"], dtype=np.float32)
    y = np.empty((B, S, D), dtype=np.float32)
    for b in range(B):
        y[b] = res.results[2 * b]["out"] + res.results[2 * b + 1]["out"] + bo
    return y


# revision 18
# speedup vs baseline: 1.2443x; 1.0203x over previous
"""Trainium2 Bass kernel: causal multi-head self-attention block (pre-LN).

Full module computed on 8 NeuronCores:
    xn = LayerNorm(x); q = xn@Wq.T+bq; k,v = xn@Wkv.T+bkv
    out = softmax(mask(q k^T / sqrt(dh))) v @ Wo.T + bo + x

Sharding: core = batch_index * 2 + head_half.  Each core handles one batch
element and 8 of the 16 heads (column-parallel QKV, row-parallel Wo), emits a
partial [S, D] output including half the residual; host sums core pairs and
adds bo.  Weights are pre-transposed and cast to bf16 on the host so they DMA
straight into the matmul-ready layout.

Shapes are hardcoded for B=4, S=2048, D=1024, H=16, DH=64.
"""

import os
import sys

import numpy as np

sys.path.insert(0, "/opt/trn_rl_repo")

B, S, D, H = 4, 2048, 1024, 16
DH = D // H            # 64
HL = H // 2            # heads per core: 8
OH = HL * DH           # per-core head features: 512
EPS = 1e-5
NEG = -30000.0         # additive mask; exp(x + NEG) underflows to 0
P = 128                # SBUF partitions
NST = S // P           # 16 s-tiles
NFT = D // P           # 8 feature tiles
NOT = OH // P          # 4 o-tiles (per-core head features)
QS = 512               # query super-tile (matmul moving free dim)
NQS = S // QS          # 4
KT_PER_QS = QS // P    # 4 k-tiles per q-super

_CACHE = {}


def _build_nc():
    import concourse.bass as bass
    import concourse.bacc as bacc
    import concourse.tile as tile
    from concourse import mybir

    f32 = mybir.dt.float32
    bf16 = mybir.dt.bfloat16
    Alu = mybir.AluOpType
    Act = mybir.ActivationFunctionType

    nc = bacc.Bacc("TRN2", target_bir_lowering=False, debug=False, num_devices=8)

    # ---- DRAM I/O (per-core shard shapes; w* pre-transposed + bf16 on host) ----
    x_d = nc.dram_tensor("x", [S, D], f32, kind="ExternalInput").ap()
    wq_d = nc.dram_tensor("wqt", [D, OH], bf16, kind="ExternalInput").ap()
    wk_d = nc.dram_tensor("wkt", [D, OH], bf16, kind="ExternalInput").ap()
    wv_d = nc.dram_tensor("wvt", [D, OH], bf16, kind="ExternalInput").ap()
    wo_d = nc.dram_tensor("wot", [OH, D], bf16, kind="ExternalInput").ap()
    bq_d = nc.dram_tensor("bq", [OH], f32, kind="ExternalInput").ap()
    bk_d = nc.dram_tensor("bk", [OH], f32, kind="ExternalInput").ap()
    bv_d = nc.dram_tensor("bv", [OH], f32, kind="ExternalInput").ap()
    g_d = nc.dram_tensor("ln_g", [D], f32, kind="ExternalInput").ap()
    b_d = nc.dram_tensor("ln_b", [D], f32, kind="ExternalInput").ap()
    pad_d = nc.dram_tensor("pad", [P, NST], f32, kind="ExternalInput").ap()
    out_d = nc.dram_tensor("out", [S, D], f32, kind="ExternalOutput").ap()
    debug = bool(os.environ.get("KERNEL_DEBUG"))
    if debug:
        dbg = {n: nc.dram_tensor(f"dbg_{n}", shp, bf16, kind="ExternalOutput").ap()
               for n, shp in (("xnT0", [P, S]), ("qT0", [P, S]), ("kT0", [P, S]),
                              ("vaug0", [P, HL * (DH + 1)]), ("oT0", [P, S]),
                              ("wqT0", [P, OH]))}

    def bcast(ap_1d, n):
        # [n] dram vector -> [P, n] partition-broadcast DMA source
        return bass.AP(tensor=ap_1d.tensor, offset=ap_1d.offset,
                       ap=[[0, P], [1, n]])

    with tile.TileContext(nc) as tc:
        with (
            tc.tile_pool(name="res", bufs=1) as res,       # resident tensors
            tc.tile_pool(name="small", bufs=4) as small,
        ):
            # ---------- constants ----------
            g_sb = res.tile([P, D], f32, tag="g_sb")
            nc.sync.dma_start(out=g_sb, in_=bcast(g_d, D))
            b_sb = res.tile([P, D], f32, tag="b_sb")
            nc.sync.dma_start(out=b_sb, in_=bcast(b_d, D))
            vb_sb = res.tile([P, OH], f32, tag="vb_sb")
            nc.sync.dma_start(out=vb_sb, in_=bcast(bv_d, OH))
            pad_sb = res.tile([P, NST], f32, tag="pad_sb")
            nc.sync.dma_start(out=pad_sb, in_=pad_d)
            bq_sb = res.tile([P, NOT], f32, tag="bq_sb")
            nc.sync.dma_start(out=bq_sb, in_=bq_d.rearrange("(t p) -> p t", p=P))
            bk_sb = res.tile([P, NOT], f32, tag="bk_sb")
            nc.sync.dma_start(out=bk_sb, in_=bk_d.rearrange("(t p) -> p t", p=P))
            eps_sb = res.tile([P, 1], f32, tag="eps_sb")
            nc.vector.memset(eps_sb, EPS)
            ident_b = res.tile([P, P], bf16, tag="ident_b")
            nc.gpsimd.memset(ident_b, 0.0)
            nc.gpsimd.affine_select(
                out=ident_b, in_=ident_b, compare_op=Alu.not_equal, fill=1.0,
                base=0, pattern=[[-1, P]], channel_multiplier=1)

            # ---------- resident big tensors ----------
            xnT = [res.tile([P, S], bf16, tag=f"xnT{j}", name=f"xnT{j}")
                   for j in range(NFT)]
            qT = [res.tile([P, S], bf16, tag=f"qT{t}", name=f"qT{t}")
                  for t in range(NOT)]
            kT = [res.tile([P, S], bf16, tag=f"kT{t}", name=f"kT{t}")
                  for t in range(NOT)]
            # V augmented with a ones column per head: [s, h*65 .. h*65+64]
            vaug = [res.tile([P, HL * (DH + 1)], bf16, tag=f"vaug{i}",
                             name=f"vaug{i}") for i in range(NST)]
            oT = [res.tile([P, S], bf16, tag=f"oT{t}", name=f"oT{t}")
                  for t in range(NOT)]
            wqT = [res.tile([P, OH], bf16, tag=f"wqT{j}", name=f"wqT{j}")
                   for j in range(NFT)]
            wkT = [res.tile([P, OH], bf16, tag=f"wkT{j}", name=f"wkT{j}")
                   for j in range(NFT)]
            wvT = [res.tile([P, OH], bf16, tag=f"wvT{j}", name=f"wvT{j}")
                   for j in range(NFT)]
            woT = [res.tile([P, D], bf16, tag=f"woT{t}", name=f"woT{t}")
                   for t in range(NOT)]

            # weights DMA straight into matmul layout (host pre-transposed)
            for j in range(NFT):
                nc.sync.dma_start(out=wqT[j], in_=wq_d[j * P:(j + 1) * P, :])
                nc.sync.dma_start(out=wkT[j], in_=wk_d[j * P:(j + 1) * P, :])
                nc.sync.dma_start(out=wvT[j], in_=wv_d[j * P:(j + 1) * P, :])
            for t in range(NOT):
                nc.sync.dma_start(out=woT[t], in_=wo_d[t * P:(t + 1) * P, :])

            # ---------- phase A: LayerNorm + transpose into xnT ----------
            with (
                tc.tile_pool(name="ld", bufs=3) as ld,
                tc.tile_pool(name="tmp", bufs=3) as tmp,
                tc.tile_pool(name="tp_psum", bufs=4, space="PSUM") as tpp,
            ):
                for st in range(NST):
                    x_t = ld.tile([P, D], f32, tag="x_ln")
                    nc.sync.dma_start(out=x_t, in_=x_d[st * P:(st + 1) * P, :])
                    stats = small.tile([P, 2, 6], f32, tag="stats")
                    for sg in range(2):
                        nc.vector.bn_stats(out=stats[:, sg, :],
                                           in_=x_t[:, sg * 512:(sg + 1) * 512])
                    mv = small.tile([P, 2], f32, tag="mv")
                    nc.vector.bn_aggr(out=mv, in_=stats)
                    rstd = small.tile([P, 1], f32, tag="rstd")
                    nc.scalar.activation(out=rstd, in_=mv[:, 1:2],
                                         func=Act.Sqrt, bias=eps_sb, scale=1.0)
                    nc.vector.reciprocal(out=rstd, in_=rstd)
                    xc = tmp.tile([P, D], f32, tag="xc")
                    nc.vector.tensor_scalar(
                        out=xc, in0=x_t, scalar1=mv[:, 0:1], scalar2=rstd,
                        op0=Alu.subtract, op1=Alu.mult)
                    xn = tmp.tile([P, D], bf16, tag="xn")
                    nc.gpsimd.tensor_mul(out=xn, in0=xc, in1=g_sb)
                    nc.gpsimd.tensor_add(out=xn, in0=xn, in1=b_sb)
                    for j in range(NFT):
                        ps = tpp.tile([P, P], bf16, tag="tp_bf")
                        nc.tensor.transpose(
                            ps, xn[:, j * P:(j + 1) * P], ident_b)
                        nc.scalar.copy(
                            out=xnT[j][:, st * P:(st + 1) * P], in_=ps)

            # ---------- phases C/D/E interleaved ----------
            # s-chunk-major projections, then per-q-super attention + output
            # projection, so PE always has dense independent work in flight.
            with (
                tc.tile_pool(name="pj_psum", bufs=3, space="PSUM") as pp,
                tc.tile_pool(name="s_psum", bufs=3, space="PSUM") as sp,
                tc.tile_pool(name="o_psum", bufs=2, space="PSUM") as op,
                tc.tile_pool(name="pt", bufs=6) as ptp,
                tc.tile_pool(name="nrm", bufs=3) as nrm,
                tc.tile_pool(name="lde", bufs=3) as lde,
                tc.tile_pool(name="tmpe", bufs=3) as tmpe,
            ):
                for st in range(NST):
                    nc.gpsimd.memset(vaug[st], 1.0)
                for c in range(NQS):
                    # --- projections for s-range [c*512, (c+1)*512) ---
                    for (wT, dst, bias) in ((wqT, qT, bq_sb), (wkT, kT, bk_sb)):
                        for t in range(NOT):
                            ps = pp.tile([P, QS], f32, tag="pj")
                            for j in range(NFT):
                                nc.tensor.matmul(
                                    ps,
                                    lhsT=wT[j][:, t * P:(t + 1) * P],
                                    rhs=xnT[j][:, c * QS:(c + 1) * QS],
                                    start=(j == 0), stop=(j == NFT - 1))
                            nc.vector.tensor_scalar_add(
                                out=dst[t][:, c * QS:(c + 1) * QS],
                                in0=ps, scalar1=bias[:, t:t + 1])
                    for st in range(c * KT_PER_QS, (c + 1) * KT_PER_QS):
                        ps = pp.tile([P, OH], f32, tag="pj")
                        for j in range(NFT):
                            nc.tensor.matmul(
                                ps,
                                lhsT=xnT[j][:, st * P:(st + 1) * P],
                                rhs=wvT[j],
                                start=(j == 0), stop=(j == NFT - 1))
                        for h in range(HL):
                            nc.vector.tensor_add(
                                out=vaug[st][:, h * (DH + 1):h * (DH + 1) + DH],
                                in0=ps[:, h * DH:(h + 1) * DH],
                                in1=vb_sb[:, h * DH:(h + 1) * DH])

                    # --- attention for q-super qs=c (needs chunks <= c) ---
                    qs = c
                    nkt = (qs + 1) * KT_PER_QS
                    for h in range(HL):
                        hq = qT[h // 2][(h % 2) * DH:(h % 2) * DH + DH, :]
                        hk = kT[h // 2][(h % 2) * DH:(h % 2) * DH + DH, :]
                        o_ps = op.tile([DH + 1, QS], f32, tag="o_ps")
                        for kt in range(nkt):
                            s_ps = sp.tile([P, QS], f32, tag="s_ps")
                            nc.tensor.matmul(
                                s_ps,
                                lhsT=hk[:, kt * P:(kt + 1) * P],
                                rhs=hq[:, qs * QS:(qs + 1) * QS],
                                start=True, stop=True, skip_group_check=True)
                            pt = ptp.tile([P, QS], bf16, tag="pt")
                            nc.scalar.activation(
                                out=pt, in_=s_ps, func=Act.Exp,
                                bias=pad_sb[:, kt:kt + 1], scale=0.125)
                            if kt >= qs * KT_PER_QS:  # diagonal region
                                nc.gpsimd.affine_select(
                                    out=pt, in_=pt,
                                    compare_op=Alu.is_ge, fill=0.0,
                                    base=qs * QS - kt * P,
                                    pattern=[[1, QS]], channel_multiplier=-1)
                            nc.tensor.matmul(
                                o_ps,
                                lhsT=vaug[kt][:, h * (DH + 1):(h + 1) * (DH + 1)],
                                rhs=pt,
                                start=(kt == 0), stop=(kt == nkt - 1),
                                skip_group_check=True)
                        den_sb = nrm.tile([1, QS], f32, tag="den_sb")
                        nc.vector.tensor_copy(den_sb, o_ps[DH:DH + 1, :])
                        dbc = nrm.tile([DH, QS], f32, tag="dbc")
                        nc.vector.reciprocal_approx_fast(
                            out=dbc[0:1, :], in_=den_sb)
                        nc.gpsimd.partition_broadcast(dbc, dbc[0:1, :])
                        nc.vector.tensor_mul(
                            out=oT[h // 2][(h % 2) * DH:(h % 2) * DH + DH,
                                           qs * QS:(qs + 1) * QS],
                            in0=o_ps[0:DH, :], in1=dbc)

                    # --- output projection for this q-super's s-tiles ---
                    for st in range(qs * KT_PER_QS, (qs + 1) * KT_PER_QS):
                        for mc in range(2):
                            ps = pp.tile([P, QS], f32, tag="pj")
                            for ot in range(NOT):
                                nc.tensor.matmul(
                                    ps,
                                    lhsT=oT[ot][:, st * P:(st + 1) * P],
                                    rhs=woT[ot][:, mc * QS:(mc + 1) * QS],
                                    start=(ot == 0), stop=(ot == NOT - 1))
                            x_sk = lde.tile([P, QS], f32, tag="x_sk")
                            nc.sync.dma_start(
                                out=x_sk,
                                in_=x_d[st * P:(st + 1) * P,
                                        mc * QS:(mc + 1) * QS])
                            y_sb = tmpe.tile([P, QS], f32, tag="y_sb")
                            nc.vector.scalar_tensor_tensor(
                                out=y_sb, in0=x_sk, scalar=0.5, in1=ps,
                                op0=Alu.mult, op1=Alu.add)
                            nc.sync.dma_start(
                                out=out_d[st * P:(st + 1) * P,
                                          mc * QS:(mc + 1) * QS],
                                in_=y_sb)

    nc.compile()
    return nc


def _get_nc():
    if "nc" not in _CACHE:
        _CACHE["nc"] = _build_nc()
    return _CACHE["nc"]


def make_in_maps(x, key_val_lengths, Wq, bq, Wkv, bkv, Wo, bo, ln_g, ln_b):
    import ml_dtypes
    bf = ml_dtypes.bfloat16

    x = np.ascontiguousarray(np.asarray(x, dtype=np.float32))
    lens = np.asarray(key_val_lengths).astype(np.int64)
    Wq = np.asarray(Wq, dtype=np.float32)
    Wkv = np.asarray(Wkv, dtype=np.float32)
    Wo = np.asarray(Wo, dtype=np.float32)
    bq = np.asarray(bq, dtype=np.float32)
    bkv = np.asarray(bkv, dtype=np.float32)
    ln_g = np.asarray(ln_g, dtype=np.float32)
    ln_b = np.asarray(ln_b, dtype=np.float32)

    in_maps = []
    for core in range(8):
        b, half = divmod(core, 2)
        sl = slice(half * OH, (half + 1) * OH)
        vsl = slice(D + half * OH, D + (half + 1) * OH)
        pad = np.where(np.arange(S) < lens[b], 0.0, NEG).astype(np.float32)
        in_maps.append({
            "x": x[b],
            "wqt": np.ascontiguousarray(Wq[sl].T.astype(bf)),
            "wkt": np.ascontiguousarray(Wkv[sl].T.astype(bf)),
            "wvt": np.ascontiguousarray(Wkv[vsl].T.astype(bf)),
            "wot": np.ascontiguousarray(Wo[:, sl].T.astype(bf)),
            "bq": np.ascontiguousarray(bq[sl]),
            "bk": np.ascontiguousarray(bkv[sl]),
            "bv": np.ascontiguousarray(bkv[vsl]),
            "ln_g": ln_g,
            "ln_b": ln_b,
            "pad": np.ascontiguousarray(pad.reshape(NST, P).T),
        })
    return in_maps


def kernel(**inputs):
    from concourse.bass_utils import run_bass_kernel_spmd

    nc = _get_nc()
    in_maps = make_in_maps(**inputs)
    res = run_bass_kernel_spmd(
        nc, in_maps, core_ids=list(range(8)),
        trace=bool(os.environ.get("KERNEL_TRACE")))
    _CACHE["last_results"] = res
    bo = np.asarray(inputs["bo"], dtype=np.float32)
    y = np.empty((B, S, D), dtype=np.float32)
    for b in range(B):
        y[b] = res.results[2 * b]["out"] + res.results[2 * b + 1]["out"] + bo
    return y
